# revision 5
# baseline (speedup 1.0000x reference)
"""Trainium2 Bass kernel for nn_MemoryDiscriminator.

Strategy (data-parallel over batch, 8 cores, 32 rows each):
  - Only outs[-1] of the reference scan is used -> no per-step outputs needed.
  - The memory-GRU state hm is batch-independent -> carried as a 33rd batch
    column through the same PSUM tiles / elementwise ops as hx.
  - gi_x = BN(e) @ w_ih_x.T is precomputed for all T as one big matmul with
    the BN affine folded into the weights (scale per contraction-dim
    partition, shift folded into the bias via a small matvec).
  - BatchNorm statistics need a global (B,T) reduction: phase A computes
    per-core per-channel mean/var on device (bn_stats/bn_aggr); the host
    combines 8x(1024,2) scalars and feeds scale/shift to phase B.
  - Transposed layout throughout: features on partitions, batch on free dim.
"""

import numpy as np

B, T, IN, H, OUT, SIM = 256, 128, 128, 1024, 256, 4
NCORES, BS = 8, 32
GOFF = [0, 128, 256, 384, 512, 640]          # gate-dim offset per tile: r0 r1 z0 z1 n0 n1
COFF = [0, 33, 66, 99, 132, 165]             # column offset of each gate block in gi row
STEP = 6 * 33                                # 198 columns per timestep in gi
EPS_BN, EPS_COS = 1e-5, 1e-8

_cache = {}


def _f16(a):
    return np.ascontiguousarray(a, dtype=np.float16)


def _f32(a):
    return np.ascontiguousarray(a, dtype=np.float32)


def _ktile(a, k, n):
    # (k*128, n) -> (128, k*n) with column-block j = rows j*128..j*128+128
    return np.ascontiguousarray(a.reshape(k, 128, n).transpose(1, 0, 2).reshape(128, k * n))


def _build_phase_a(zero_bemb):
    from concourse import bacc
    import concourse.mybir as mybir
    import concourse.tile as tile

    f32, f16 = mybir.dt.float32, mybir.dt.float16
    AF = mybir.ActivationFunctionType
    OP = mybir.AluOpType

    nc = bacc.Bacc()
    xT = nc.declare_dram_parameter("xT", [128, BS * T], f16, isOutput=False)
    WembT = nc.declare_dram_parameter("WembT", [128, H], f16, isOutput=False)
    bemb = nc.declare_dram_parameter("bemb", [128, 8], f32, isOutput=False)
    bnout = nc.declare_dram_parameter("bnout", [128, 8, 2], f32, isOutput=True)

    with tile.TileContext(nc) as tc:
        with tc.tile_pool(name="consts", bufs=1) as consts, \
             tc.tile_pool(name="work", bufs=3) as work, \
             tc.tile_pool(name="stat", bufs=1) as statp, \
             tc.tile_pool(name="ps", bufs=3, space="PSUM") as psum:
            xt = consts.tile([128, BS * T], f16)
            wt = consts.tile([128, H], f16)
            nc.sync.dma_start(out=xt[:], in_=xT[:])
            nc.sync.dma_start(out=wt[:], in_=WembT[:])
            if not zero_bemb:
                bt = consts.tile([128, 8], f32)
                nc.sync.dma_start(out=bt[:], in_=bemb[:])
            out_sb = statp.tile([128, 8, 2], mybir.dt.float32)
            NCH = 8
            CH = BS * T // NCH                     # 512
            for m in range(8):
                stats = statp.tile([128, NCH, 6], mybir.dt.float32)
                for ch in range(NCH):
                    pe = psum.tile([128, CH], mybir.dt.float32)
                    nc.tensor.matmul(pe[:], wt[:, m * 128:(m + 1) * 128],
                                     xt[:, ch * CH:(ch + 1) * CH], start=True, stop=True)
                    e_sb = work.tile([128, CH], mybir.dt.float32)
                    if zero_bemb:
                        rl = work.tile([128, CH], mybir.dt.float32)
                        nc.scalar.activation(rl[:], pe[:], AF.Relu, bias=0.0, scale=0.8)
                        nc.vector.scalar_tensor_tensor(e_sb[:], pe[:], 0.2, rl[:],
                                                       op0=OP.mult, op1=OP.add)
                    else:
                        lin = work.tile([128, CH], mybir.dt.float32)
                        nc.scalar.activation(lin[:], pe[:], AF.Identity,
                                             bias=bt[:, m:m + 1], scale=1.0)
                        nc.vector.scalar_tensor_tensor(e_sb[:], lin[:], 0.2, lin[:],
                                                       op0=OP.mult, op1=OP.max)
                    nc.vector.bn_stats(stats[:, ch, :], e_sb[:])
                nc.vector.bn_aggr(out_sb[:, m, :], stats[:])
            nc.sync.dma_start(out=bnout[:], in_=out_sb[:])
    nc.finalize()
    return nc


def _build_phase_b(zero_bemb, has_bhn):
    from concourse import bacc
    import concourse.mybir as mybir
    import concourse.tile as tile

    f32, f16 = mybir.dt.float32, mybir.dt.float16
    AF = mybir.ActivationFunctionType
    OP = mybir.AluOpType

    nc = bacc.Bacc()
    dp = nc.declare_dram_parameter
    xT = dp("xT", [128, BS * T], f16, isOutput=False)
    WembT = dp("WembT", [128, H], f16, isOutput=False)
    bemb = dp("bemb", [128, 8], f32, isOutput=False)
    scl = dp("scl", [128, 8], f32, isOutput=False)       # BN scale per channel
    shf16 = dp("shf16", [128, 8], f16, isOutput=False)   # BN shift (fp16)
    wihxT = dp("wihxT", [128, 8 * 768], f16, isOutput=False)
    bgx = dp("bgx", [128, 6], f32, isOutput=False)
    wihmT = dp("wihmT", [128, 8 * 768], f16, isOutput=False)
    bgm = dp("bgm", [128, 6], f32, isOutput=False)
    memT = dp("memT", [128, 8 * T], f16, isOutput=False)
    whhxT = dp("whhxT", [128, 2 * 768], f16, isOutput=False)
    whhmT = dp("whhmT", [128, 2 * 768], f16, isOutput=False)
    ident = dp("ident", [128, 128], f16, isOutput=False)
    WsxT = dp("WsxT", [128, 2 * SIM], f16, isOutput=False)
    WsmT = dp("WsmT", [128, 2 * SIM], f16, isOutput=False)
    bs4 = dp("bs4", [SIM, 2], f32, isOutput=False)       # col0 = b_sx, col1 = b_sm
    ones4 = dp("ones4", [SIM, 1], f32, isOutput=False)
    ones128 = dp("ones128", [1, 128], f32, isOutput=False)
    if has_bhn:
        bhn = dp("bhn", [1, 512], f32, isOutput=False)  # [bhh_x_n (256) | bhh_m_n (256)]
        onesb = dp("onesb", [1, BS], f32, isOutput=False)
    outT = dp("outT", [128, 2, BS], f32, isOutput=True)

    with tile.TileContext(nc) as tc:
        with tc.tile_pool(name="consts", bufs=1) as consts, \
             tc.tile_pool(name="gi", bufs=1) as gip, \
             tc.tile_pool(name="state", bufs=3) as statep, \
             tc.tile_pool(name="work", bufs=3) as work:
            # ---- load inputs ----
            def cl(t_, src):
                tt = consts.tile(list(src.shape), src.dtype, tag=t_)
                nc.sync.dma_start(out=tt[:], in_=src[:])
                return tt

            xt = cl("xt", xT); wemb = cl("wemb", WembT)
            sclt = cl("sclt", scl); shft = cl("shft", shf16)
            wix = cl("wix", wihxT); bgxt = cl("bgxt", bgx)
            wim = cl("wim", wihmT); bgmt = cl("bgmt", bgm)
            memt = cl("memt", memT)
            whx = cl("whx", whhxT); whm = cl("whm", whhmT)
            idt = cl("idt", ident)
            wsx = cl("wsx", WsxT); wsm = cl("wsm", WsmT)
            bst = cl("bst", bs4); o4 = cl("o4", ones4); o128 = cl("o128", ones128)
            if not zero_bemb:
                bembt = cl("bembt", bemb)
            if has_bhn:
                bhnt = cl("bhnt", bhn); onbt = cl("onbt", onesb)

            gi_sb = gip.tile([128, T * STEP], f16)      # (128, 25344)
            gi3 = gi_sb[:].rearrange("p (t c) -> p t c", c=STEP)

            # ---- B1: fold W2 = wihxT * scale (per-partition) ----
            W2 = consts.tile([128, 8 * 768], f16)
            for k in range(8):
                nc.vector.tensor_scalar_mul(W2[:, k * 768:(k + 1) * 768],
                                            wix[:, k * 768:(k + 1) * 768],
                                            sclt[:, k:k + 1])

            with tc.tile_pool(name="pps", bufs=3, space="PSUM") as pps, \
                 tc.tile_pool(name="gps", bufs=2, space="PSUM") as gps, \
                 tc.tile_pool(name="cps", bufs=1, space="PSUM") as cps:
                # ---- B2: gi bias tiles: biasg[:, g] = shift @ w_ih_x.T |tile g  + bgx[:, g]
                biasg = consts.tile([128, 6], mybir.dt.float32)
                pc = cps.tile([128, 6], mybir.dt.float32)
                for g in range(6):
                    for k in range(8):
                        nc.tensor.matmul(pc[:, g:g + 1],
                                         wix[:, k * 768 + GOFF[g]: k * 768 + GOFF[g] + 128],
                                         shft[:, k:k + 1],
                                         start=(k == 0), stop=(k == 7),
                                         skip_group_check=True)
                for g in range(6):
                    nc.vector.tensor_add(biasg[:, g:g + 1], pc[:, g:g + 1], bgxt[:, g:g + 1])

                # ---- B3: embed + gi_x streaming over 8 chunks of 512 cols (16 t each) ----
                NCH = 8
                CH = BS * T // NCH                      # 512
                TCH = CH // BS                          # 16 timesteps per chunk
                for ch in range(NCH):
                    e16 = work.tile([128, 8 * CH], f16, tag="e16")
                    for m in range(8):
                        pe = pps.tile([128, CH], mybir.dt.float32)
                        nc.tensor.matmul(pe[:], wemb[:, m * 128:(m + 1) * 128],
                                         xt[:, ch * CH:(ch + 1) * CH], start=True, stop=True)
                        if zero_bemb:
                            rl = work.tile([128, CH], mybir.dt.float32, tag="rl")
                            nc.scalar.activation(rl[:], pe[:], AF.Relu, bias=0.0, scale=0.8)
                            nc.vector.scalar_tensor_tensor(e16[:, m * CH:(m + 1) * CH],
                                                           pe[:], 0.2, rl[:],
                                                           op0=OP.mult, op1=OP.add)
                        else:
                            lin = work.tile([128, CH], mybir.dt.float32, tag="rl")
                            nc.scalar.activation(lin[:], pe[:], AF.Identity,
                                                 bias=bembt[:, m:m + 1], scale=1.0)
                            nc.vector.scalar_tensor_tensor(e16[:, m * CH:(m + 1) * CH],
                                                           lin[:], 0.2, lin[:],
                                                           op0=OP.mult, op1=OP.max)
                    for g in range(6):
                        pg = gps.tile([128, CH], mybir.dt.float32)
                        for k in range(8):
                            nc.tensor.matmul(pg[:], W2[:, k * 768 + GOFF[g]: k * 768 + GOFF[g] + 128],
                                             e16[:, k * CH:(k + 1) * CH],
                                             start=(k == 0), stop=(k == 7))
                        src = pg[:].rearrange("p (t b) -> p t b", b=BS)
                        dst = gi3[:, ch * TCH:(ch + 1) * TCH, COFF[g]:COFF[g] + BS]
                        nc.scalar.activation(dst, src, AF.Identity,
                                             bias=biasg[:, g:g + 1], scale=1.0)

                # ---- B4: gi_m into column 32 of each gate block ----
                for g in range(6):
                    pg = gps.tile([128, T], mybir.dt.float32, tag="pgm")
                    for k in range(8):
                        nc.tensor.matmul(pg[:], wim[:, k * 768 + GOFF[g]: k * 768 + GOFF[g] + 128],
                                         memt[:, k * T:(k + 1) * T],
                                         start=(k == 0), stop=(k == 7))
                    src = pg[:].rearrange("p (t b) -> p t b", b=1)
                    dst = gi3[:, :, COFF[g] + BS:COFF[g] + BS + 1]
                    nc.scalar.activation(dst, src, AF.Identity,
                                         bias=bgmt[:, g:g + 1], scale=1.0)

            # ---- scan ----
            with tc.tile_pool(name="rzp", bufs=2, space="PSUM") as rzp, \
                 tc.tile_pool(name="npp", bufs=2, space="PSUM") as npp, \
                 tc.tile_pool(name="fin", bufs=1, space="PSUM") as finp:
                hxm = statep.tile([128, 66], f16, tag="hxm")
                nc.gpsimd.memset(hxm[:], 0.0)
                for t in range(T):
                    prz = rzp.tile([128, 132], mybir.dt.float32)
                    pn = npp.tile([128, 66], mybir.dt.float32)
                    # identity first (no data dependence on state): start=True clears bank
                    nc.tensor.matmul(prz[:], idt[:], gi3[:, t, 0:132],
                                     start=True, stop=False, skip_group_check=True)
                    # rz gate tiles: g=0..3
                    for g in range(4):
                        for k in range(2):
                            nc.tensor.matmul(prz[:, g * 33:g * 33 + 32],
                                             whx[:, k * 768 + GOFF[g]: k * 768 + GOFF[g] + 128],
                                             hxm[:, k * 33:k * 33 + 32],
                                             start=False, stop=False, skip_group_check=True)
                            nc.tensor.matmul(prz[:, g * 33 + 32:g * 33 + 33],
                                             whm[:, k * 768 + GOFF[g]: k * 768 + GOFF[g] + 128],
                                             hxm[:, k * 33 + 32:k * 33 + 33],
                                             start=False, stop=(g == 3 and k == 1),
                                             skip_group_check=True)
                    # n gate tiles: g=4,5
                    first = True
                    for g in range(4, 6):
                        gg = g - 4
                        for k in range(2):
                            nc.tensor.matmul(pn[:, gg * 33:gg * 33 + 32],
                                             whx[:, k * 768 + GOFF[g]: k * 768 + GOFF[g] + 128],
                                             hxm[:, k * 33:k * 33 + 32],
                                             start=first, stop=False, skip_group_check=True)
                            first = False
                            nc.tensor.matmul(pn[:, gg * 33 + 32:gg * 33 + 33],
                                             whm[:, k * 768 + GOFF[g]: k * 768 + GOFF[g] + 128],
                                             hxm[:, k * 33 + 32:k * 33 + 33],
                                             start=False, stop=(g == 5 and k == 1),
                                             skip_group_check=True)
                    if has_bhn:
                        for kk in range(2):
                            nc.tensor.matmul(pn[:, kk * 33:kk * 33 + 32],
                                             bhnt[0:1, kk * 128:(kk + 1) * 128],
                                             onbt[0:1, :], start=False, stop=False,
                                             skip_group_check=True)
                            nc.tensor.matmul(pn[:, kk * 33 + 32:kk * 33 + 33],
                                             bhnt[0:1, 256 + kk * 128:256 + (kk + 1) * 128][:, 0:128],
                                             onbt[0:1, 0:1], start=False, stop=False,
                                             skip_group_check=True)
                    rz_sb = work.tile([128, 132], mybir.dt.float32, tag="rz")
                    nc.scalar.activation(rz_sb[:], prz[:], AF.Sigmoid)
                    t1 = work.tile([128, 66], mybir.dt.float32, tag="t1")
                    nc.vector.tensor_mul(t1[:], rz_sb[:, 0:66], pn[:])
                    t2 = work.tile([128, 66], mybir.dt.float32, tag="t2")
                    nc.vector.tensor_add(t2[:], t1[:], gi3[:, t, 132:198])
                    n_sb = work.tile([128, 66], mybir.dt.float32, tag="nsb")
                    nc.scalar.activation(n_sb[:], t2[:], AF.Tanh)
                    dd = work.tile([128, 66], mybir.dt.float32, tag="dd")
                    nc.vector.tensor_sub(dd[:], hxm[:], n_sb[:])
                    mm_ = work.tile([128, 66], mybir.dt.float32, tag="mm")
                    nc.vector.tensor_mul(mm_[:], rz_sb[:, 66:132], dd[:])
                    hxm = statep.tile([128, 66], f16, tag="hxm")
                    nc.vector.tensor_add(hxm[:], n_sb[:], mm_[:])

                # ---- final gate ----
                pq = finp.tile([SIM, 34], mybir.dt.float32, tag="pq")
                for k in range(2):
                    nc.tensor.matmul(pq[:, 0:33], wsx[:, k * SIM:(k + 1) * SIM],
                                     hxm[:, k * 33:(k + 1) * 33],
                                     start=(k == 0), stop=False, skip_group_check=True)
                for k in range(2):
                    nc.tensor.matmul(pq[:, 33:34], wsm[:, k * SIM:(k + 1) * SIM],
                                     hxm[:, k * 33 + 32:k * 33 + 33],
                                     start=False, stop=(k == 1), skip_group_check=True)
                q_sb = work.tile([SIM, 34], mybir.dt.float32, tag="qsb")
                nc.scalar.activation(q_sb[:, 0:33], pq[:, 0:33], AF.Identity,
                                     bias=bst[:, 0:1], scale=1.0)
                nc.scalar.activation(q_sb[:, 33:34], pq[:, 33:34], AF.Identity,
                                     bias=bst[:, 1:2], scale=1.0)
                p_sb = work.tile([SIM, 67], mybir.dt.float32, tag="psb")
                nc.vector.tensor_scalar_mul(p_sb[:, 0:33], q_sb[:, 0:33], q_sb[:, 33:34])
                nc.vector.tensor_mul(p_sb[:, 33:67], q_sb[:], q_sb[:])
                cs = finp.tile([1, 67], mybir.dt.float32, tag="cs")
                nc.tensor.matmul(cs[:], o4[:], p_sb[:], start=True, stop=True,
                                 skip_group_check=True)
                s_sb = work.tile([1, 34], mybir.dt.float32, tag="ssb")
                nc.scalar.activation(s_sb[:], cs[0:1, 33:67], AF.Sqrt)
                nc.vector.tensor_scalar_max(s_sb[:], s_sb[:], EPS_COS)
                den = work.tile([1, 33], mybir.dt.float32, tag="den")
                nc.vector.tensor_scalar_mul(den[:], s_sb[:, 0:33], s_sb[:, 33:34])
                nc.vector.reciprocal(den[:], den[:])
                rat = work.tile([1, 33], mybir.dt.float32, tag="rat")
                nc.vector.tensor_mul(rat[:], cs[0:1, 0:33], den[:])
                g_sb = work.tile([1, 33], mybir.dt.float32, tag="gsb")
                nc.scalar.activation(g_sb[:], rat[:], AF.Sigmoid)
                gbc = finp.tile([128, BS], mybir.dt.float32, tag="gbc")
                nc.tensor.matmul(gbc[:], o128[:], g_sb[0:1, 0:BS], start=True, stop=True,
                                 skip_group_check=True)
                hm32 = work.tile([128, 2], mybir.dt.float32, tag="hm32")
                hmv = hxm[:].rearrange("p (k c) -> p k c", c=33)[:, :, 32:33]
                nc.scalar.activation(hm32[:].rearrange("p (k c) -> p k c", c=1), hmv,
                                     AF.Identity, bias=0.0, scale=1.0)
                out_sb = work.tile([128, 2, BS], mybir.dt.float32, tag="outsb")
                for k in range(2):
                    ddk = work.tile([128, BS], mybir.dt.float32, tag="ddk")
                    nc.vector.tensor_scalar_sub(ddk[:], hxm[:, k * 33:k * 33 + 32],
                                                hm32[:, k:k + 1])
                    ppk = work.tile([128, BS], mybir.dt.float32, tag="ppk")
                    nc.vector.tensor_mul(ppk[:], gbc[:], ddk[:])
                    nc.vector.tensor_scalar_add(out_sb[:, k, :], ppk[:], hm32[:, k:k + 1])
                nc.sync.dma_start(out=outT[:], in_=out_sb[:])
    nc.finalize()
    return nc


def _prep_inputs(inputs):
    x = _f32(inputs["x"])
    W_emb = _f32(inputs["W_emb"]); b_emb = _f32(inputs["b_emb"])
    gamma = _f32(inputs["gamma"]); beta = _f32(inputs["beta"])
    mem = _f32(inputs["memory"])[0]
    w_ih_x = _f32(inputs["w_ih_x"]); w_hh_x = _f32(inputs["w_hh_x"])
    b_ih_x = _f32(inputs["b_ih_x"]); b_hh_x = _f32(inputs["b_hh_x"])
    w_ih_m = _f32(inputs["w_ih_m"]); w_hh_m = _f32(inputs["w_hh_m"])
    b_ih_m = _f32(inputs["b_ih_m"]); b_hh_m = _f32(inputs["b_hh_m"])
    W_sx = _f32(inputs["W_sx"]); b_sx = _f32(inputs["b_sx"])
    W_sm = _f32(inputs["W_sm"]); b_sm = _f32(inputs["b_sm"])

    zero_bemb = not np.any(b_emb)
    has_bhn = bool(np.any(b_hh_x[512:]) or np.any(b_hh_m[512:]))

    per_core = []
    WembT = _f16(W_emb.T)
    bemb_t = _f32(b_emb.reshape(8, 128).T)
    for c in range(NCORES):
        xc = x[c * BS:(c + 1) * BS]                       # (32,T,IN)
        xTc = _f16(xc.transpose(2, 1, 0).reshape(IN, T * BS))
        per_core.append({"xT": xTc, "WembT": WembT, "bemb": bemb_t})

    shared = dict(
        WembT=WembT, bemb=bemb_t,
        wihxT=_f16(_ktile(w_ih_x.T, 8, 768)),
        wihmT=_f16(_ktile(w_ih_m.T, 8, 768)),
        memT=_f16(_ktile(mem.T, 8, T)),
        whhxT=_f16(_ktile(w_hh_x.T, 2, 768)),
        whhmT=_f16(_ktile(w_hh_m.T, 2, 768)),
        ident=_f16(np.eye(128)),
        WsxT=_f16(_ktile(W_sx.T, 2, SIM)),
        WsmT=_f16(_ktile(W_sm.T, 2, SIM)),
        bs4=_f32(np.stack([b_sx, b_sm], axis=1)),
        ones4=_f32(np.ones((SIM, 1))),
        ones128=_f32(np.ones((1, 128))),
    )
    bgx = np.empty((128, 6), np.float32)
    bgm = np.empty((128, 6), np.float32)
    for g in range(6):
        sl = slice(GOFF[g], GOFF[g] + 128)
        if g < 4:
            bgx[:, g] = b_ih_x[sl] + b_hh_x[sl]
            bgm[:, g] = b_ih_m[sl] + b_hh_m[sl]
        else:
            bgx[:, g] = b_ih_x[sl]
            bgm[:, g] = b_ih_m[sl]
    shared["bgx"] = _f32(bgx); shared["bgm"] = _f32(bgm)
    if has_bhn:
        shared["bhn"] = _f32(np.concatenate([b_hh_x[512:768], b_hh_m[512:768]]).reshape(1, -1))
        shared["onesb"] = _f32(np.ones((1, BS)))
    meta = dict(zero_bemb=zero_bemb, has_bhn=has_bhn, gamma=gamma, beta=beta)
    return per_core, shared, meta


def _combine_stats(bn_results, gamma, beta):
    # bn_results: list of (128,8,2) per core; channel h = m*128+p
    means = np.stack([r.reshape(128, 8, 2)[:, :, 0].T.reshape(H) for r in bn_results])
    vars_ = np.stack([r.reshape(128, 8, 2)[:, :, 1].T.reshape(H) for r in bn_results])
    mean = means.mean(0)
    var = (vars_ + means ** 2).mean(0) - mean ** 2
    scale = gamma / np.sqrt(var + EPS_BN)
    shift = beta - mean * scale
    return scale, shift


def get_programs(zero_bemb, has_bhn):
    key = ("progs", zero_bemb, has_bhn)
    if key not in _cache:
        _cache[key] = (_build_phase_a(zero_bemb), _build_phase_b(zero_bemb, has_bhn))
    return _cache[key]


def kernel(**inputs) -> np.ndarray:
    from concourse.bass_utils import run_bass_kernel_spmd

    per_core, shared, meta = _prep_inputs(inputs)
    nc_a, nc_b = get_programs(meta["zero_bemb"], meta["has_bhn"])
    core_ids = list(range(NCORES))

    in_a = [{"xT": pc["xT"], "WembT": pc["WembT"], "bemb": pc["bemb"]}
            for pc in per_core]
    res_a = run_bass_kernel_spmd(nc_a, in_a, core_ids=core_ids).results
    scale, shift = _combine_stats([r["bnout"] for r in res_a],
                                  meta["gamma"], meta["beta"])

    scl_t = _f32(scale.reshape(8, 128).T)
    shf_t = _f16(shift.reshape(8, 128).T)
    in_b = []
    for c in range(NCORES):
        m = {"xT": per_core[c]["xT"], "scl": scl_t, "shf16": shf_t}
        m.update(shared)
        in_b.append(m)
    res_b = run_bass_kernel_spmd(nc_b, in_b, core_ids=core_ids).results

    out = np.empty((B, OUT), np.float32)
    for c in range(NCORES):
        o = res_b[c]["outT"].reshape(128, 2, BS)
        out[c * BS:(c + 1) * BS] = o.transpose(2, 1, 0).reshape(BS, OUT)
    return out


# revision 8
# speedup vs baseline: 1.0168x; 1.0168x over previous
"""Trainium2 Bass kernel for nn_MemoryDiscriminator.

Strategy (data-parallel over batch, 8 cores, 32 rows each):
  - Only outs[-1] of the reference scan is used -> no per-step outputs needed.
  - The memory-GRU state hm is batch-independent -> carried as a 33rd batch
    column through the same PSUM tiles / elementwise ops as hx.
  - gi_x = BN(e) @ w_ih_x.T is precomputed for all T as one big matmul with
    the BN affine folded into the weights (scale per contraction-dim
    partition, shift folded into the bias via a small matvec).
  - BatchNorm statistics need a global (B,T) reduction: phase A computes
    per-core per-channel mean/var on device (bn_stats/bn_aggr); the host
    combines 8x(1024,2) scalars and feeds scale/shift to phase B.
  - Transposed layout throughout: features on partitions, batch on free dim.
"""

import numpy as np

B, T, IN, H, OUT, SIM = 256, 128, 128, 1024, 256, 4
NCORES, BS = 8, 32
GOFF = [0, 128, 256, 384, 512, 640]          # gate-dim offset per tile: r0 r1 z0 z1 n0 n1
COFF = [0, 33, 66, 99, 132, 165]             # column offset of each gate block in gi row
STEP = 6 * 33                                # 198 columns per timestep in gi
EPS_BN, EPS_COS = 1e-5, 1e-8

_cache = {}


def _f16(a):
    return np.ascontiguousarray(a, dtype=np.float16)


def _f32(a):
    return np.ascontiguousarray(a, dtype=np.float32)


def _ktile(a, k, n):
    # (k*128, n) -> (128, k*n) with column-block j = rows j*128..j*128+128
    return np.ascontiguousarray(a.reshape(k, 128, n).transpose(1, 0, 2).reshape(128, k * n))


def _build_phase_a(zero_bemb):
    from concourse import bacc
    import concourse.mybir as mybir
    import concourse.tile as tile

    f32, f16 = mybir.dt.float32, mybir.dt.float16
    AF = mybir.ActivationFunctionType
    OP = mybir.AluOpType

    nc = bacc.Bacc()
    xT = nc.declare_dram_parameter("xT", [128, BS * T], f16, isOutput=False)
    WembT = nc.declare_dram_parameter("WembT", [128, H], f16, isOutput=False)
    bemb = nc.declare_dram_parameter("bemb", [128, 8], f32, isOutput=False)
    bnout = nc.declare_dram_parameter("bnout", [128, 8, 2], f32, isOutput=True)

    with tile.TileContext(nc) as tc:
        with tc.tile_pool(name="consts", bufs=1) as consts, \
             tc.tile_pool(name="work", bufs=3) as work, \
             tc.tile_pool(name="stat", bufs=1) as statp, \
             tc.tile_pool(name="ps", bufs=3, space="PSUM") as psum:
            xt = consts.tile([128, BS * T], f16)
            wt = consts.tile([128, H], f16)
            nc.sync.dma_start(out=xt[:], in_=xT[:])
            nc.sync.dma_start(out=wt[:], in_=WembT[:])
            if not zero_bemb:
                bt = consts.tile([128, 8], f32)
                nc.sync.dma_start(out=bt[:], in_=bemb[:])
            out_sb = statp.tile([128, 8, 2], mybir.dt.float32)
            NCH = 8
            CH = BS * T // NCH                     # 512
            for m in range(8):
                stats = statp.tile([128, NCH, 6], mybir.dt.float32)
                for ch in range(NCH):
                    pe = psum.tile([128, CH], mybir.dt.float32)
                    nc.tensor.matmul(pe[:], wt[:, m * 128:(m + 1) * 128],
                                     xt[:, ch * CH:(ch + 1) * CH], start=True, stop=True)
                    e_sb = work.tile([128, CH], mybir.dt.float32)
                    if zero_bemb:
                        rl = work.tile([128, CH], mybir.dt.float32)
                        nc.scalar.activation(rl[:], pe[:], AF.Relu, bias=0.0, scale=0.8)
                        nc.vector.scalar_tensor_tensor(e_sb[:], pe[:], 0.2, rl[:],
                                                       op0=OP.mult, op1=OP.add)
                    else:
                        lin = work.tile([128, CH], mybir.dt.float32)
                        nc.scalar.activation(lin[:], pe[:], AF.Identity,
                                             bias=bt[:, m:m + 1], scale=1.0)
                        nc.vector.scalar_tensor_tensor(e_sb[:], lin[:], 0.2, lin[:],
                                                       op0=OP.mult, op1=OP.max)
                    nc.vector.bn_stats(stats[:, ch, :], e_sb[:])
                nc.vector.bn_aggr(out_sb[:, m, :], stats[:])
            nc.sync.dma_start(out=bnout[:], in_=out_sb[:])
    nc.finalize()
    return nc


def _build_phase_b(zero_bemb, has_bhn):
    from concourse import bacc
    import concourse.mybir as mybir
    import concourse.tile as tile

    f32, f16 = mybir.dt.float32, mybir.dt.float16
    AF = mybir.ActivationFunctionType
    OP = mybir.AluOpType

    nc = bacc.Bacc()
    dp = nc.declare_dram_parameter
    xT = dp("xT", [128, BS * T], f16, isOutput=False)
    WembT = dp("WembT", [128, H], f16, isOutput=False)
    bemb = dp("bemb", [128, 8], f32, isOutput=False)
    scl = dp("scl", [128, 8], f32, isOutput=False)       # BN scale per channel
    shf16 = dp("shf16", [128, 8], f16, isOutput=False)   # BN shift (fp16)
    wihxT = dp("wihxT", [128, 8 * 768], f16, isOutput=False)
    bgx = dp("bgx", [128, 6], f32, isOutput=False)
    wihmT = dp("wihmT", [128, 8 * 768], f16, isOutput=False)
    bgm = dp("bgm", [128, 6], f32, isOutput=False)
    memT = dp("memT", [128, 8 * T], f16, isOutput=False)
    whhxT = dp("whhxT", [128, 2 * 768], f16, isOutput=False)
    whhmT = dp("whhmT", [128, 2 * 768], f16, isOutput=False)
    ident = dp("ident", [128, 128], f16, isOutput=False)
    WsxT = dp("WsxT", [128, 2 * SIM], f16, isOutput=False)
    WsmT = dp("WsmT", [128, 2 * SIM], f16, isOutput=False)
    bs4 = dp("bs4", [SIM, 2], f32, isOutput=False)       # col0 = b_sx, col1 = b_sm
    ones4 = dp("ones4", [SIM, 1], f32, isOutput=False)
    ones128 = dp("ones128", [1, 128], f32, isOutput=False)
    if has_bhn:
        bhn = dp("bhn", [1, 512], f32, isOutput=False)  # [bhh_x_n (256) | bhh_m_n (256)]
        onesb = dp("onesb", [1, BS], f32, isOutput=False)
    outT = dp("outT", [128, 2, BS], f32, isOutput=True)

    with tile.TileContext(nc) as tc:
        with tc.tile_pool(name="consts", bufs=1) as consts, \
             tc.tile_pool(name="gi", bufs=1) as gip, \
             tc.tile_pool(name="state", bufs=3) as statep, \
             tc.tile_pool(name="work", bufs=3) as work:
            # ---- load inputs ----
            def cl(t_, src):
                tt = consts.tile(list(src.shape), src.dtype, tag=t_)
                nc.sync.dma_start(out=tt[:], in_=src[:])
                return tt

            xt = cl("xt", xT); wemb = cl("wemb", WembT)
            sclt = cl("sclt", scl); shft = cl("shft", shf16)
            wix = cl("wix", wihxT); bgxt = cl("bgxt", bgx)
            wim = cl("wim", wihmT); bgmt = cl("bgmt", bgm)
            memt = cl("memt", memT)
            whx = cl("whx", whhxT); whm = cl("whm", whhmT)
            idt = cl("idt", ident)
            wsx = cl("wsx", WsxT); wsm = cl("wsm", WsmT)
            bst = cl("bst", bs4); o4 = cl("o4", ones4); o128 = cl("o128", ones128)
            if not zero_bemb:
                bembt = cl("bembt", bemb)
            if has_bhn:
                bhnt = cl("bhnt", bhn); onbt = cl("onbt", onesb)

            gi_sb = gip.tile([128, T * STEP], f16)      # (128, 25344)
            gi3 = gi_sb[:].rearrange("p (t c) -> p t c", c=STEP)

            # ---- B1: fold W2 = wihxT * scale (per-partition) ----
            W2 = consts.tile([128, 8 * 768], f16)
            for k in range(8):
                nc.vector.tensor_scalar_mul(W2[:, k * 768:(k + 1) * 768],
                                            wix[:, k * 768:(k + 1) * 768],
                                            sclt[:, k:k + 1])

            with tc.tile_pool(name="rzp", bufs=2, space="PSUM") as rzp, \
                 tc.tile_pool(name="npp", bufs=2, space="PSUM") as npp:
                hxm = statep.tile([128, 66], f16, tag="hxm")
                nc.gpsimd.memset(hxm[:], 0.0)

                def scan_step(t):
                    nonlocal hxm
                    prz = rzp.tile([128, 132], mybir.dt.float32)
                    pn = npp.tile([128, 66], mybir.dt.float32)
                    # identity first (no data dependence on state): start=True clears bank
                    nc.tensor.matmul(prz[:], idt[:], gi3[:, t, 0:132],
                                     start=True, stop=False, skip_group_check=True)
                    # k-major: all k=0 matmuls first (they only need hxm cols 0:33)
                    first_n = True
                    for k in range(2):
                        for g in range(4):
                            nc.tensor.matmul(prz[:, g * 33:g * 33 + 32],
                                             whx[:, k * 768 + GOFF[g]: k * 768 + GOFF[g] + 128],
                                             hxm[:, k * 33:k * 33 + 32],
                                             start=False, stop=False, skip_group_check=True)
                            nc.tensor.matmul(prz[:, g * 33 + 32:g * 33 + 33],
                                             whm[:, k * 768 + GOFF[g]: k * 768 + GOFF[g] + 128],
                                             hxm[:, k * 33 + 32:k * 33 + 33],
                                             start=False, stop=(g == 3 and k == 1),
                                             skip_group_check=True)
                        for g in range(4, 6):
                            gg = g - 4
                            nc.tensor.matmul(pn[:, gg * 33:gg * 33 + 32],
                                             whx[:, k * 768 + GOFF[g]: k * 768 + GOFF[g] + 128],
                                             hxm[:, k * 33:k * 33 + 32],
                                             start=first_n, stop=False, skip_group_check=True)
                            first_n = False
                            nc.tensor.matmul(pn[:, gg * 33 + 32:gg * 33 + 33],
                                             whm[:, k * 768 + GOFF[g]: k * 768 + GOFF[g] + 128],
                                             hxm[:, k * 33 + 32:k * 33 + 33],
                                             start=False, stop=(g == 5 and k == 1),
                                             skip_group_check=True)
                    if has_bhn:
                        for kk in range(2):
                            nc.tensor.matmul(pn[:, kk * 33:kk * 33 + 32],
                                             bhnt[0:1, kk * 128:(kk + 1) * 128],
                                             onbt[0:1, :], start=False, stop=False,
                                             skip_group_check=True)
                            nc.tensor.matmul(pn[:, kk * 33 + 32:kk * 33 + 33],
                                             bhnt[0:1, 256 + kk * 128:256 + (kk + 1) * 128],
                                             onbt[0:1, 0:1], start=False, stop=False,
                                             skip_group_check=True)
                    # critical chain: sig_r -> t1 -> t2 -> tanh -> nz -> hxm'
                    rs = work.tile([128, 66], mybir.dt.float32, tag="rs")
                    nc.scalar.activation(rs[:], prz[:, 0:66], AF.Sigmoid)
                    zc = work.tile([128, 66], mybir.dt.float32, tag="zc")
                    nc.scalar.activation(zc[:], prz[:, 66:132], AF.Sigmoid, scale=-1.0)
                    t1 = work.tile([128, 66], mybir.dt.float32, tag="t1")
                    nc.vector.tensor_mul(t1[:], rs[:], pn[:])
                    t2 = work.tile([128, 66], mybir.dt.float32, tag="t2")
                    nc.vector.tensor_add(t2[:], t1[:], gi3[:, t, 132:198])
                    # off-chain: u = hxm - zc*hxm  (runs during tanh)
                    zh = work.tile([128, 66], mybir.dt.float32, tag="zh")
                    nc.vector.tensor_mul(zh[:], zc[:], hxm[:])
                    u = work.tile([128, 66], mybir.dt.float32, tag="u")
                    nc.vector.tensor_sub(u[:], hxm[:], zh[:])
                    n_sb = work.tile([128, 66], mybir.dt.float32, tag="nsb")
                    nc.scalar.activation(n_sb[:], t2[:], AF.Tanh)
                    nz = work.tile([128, 66], mybir.dt.float32, tag="nz")
                    nc.vector.tensor_mul(nz[:], zc[:], n_sb[:])
                    hxm = statep.tile([128, 66], f16, tag="hxm")
                    # k0 half first so next step's k=0 matmuls can start earlier
                    nc.vector.tensor_add(hxm[:, 0:33], u[:, 0:33], nz[:, 0:33])
                    nc.vector.tensor_add(hxm[:, 33:66], u[:, 33:66], nz[:, 33:66])

                with tc.tile_pool(name="pps", bufs=2, space="PSUM") as pps, \
                     tc.tile_pool(name="gps", bufs=2, space="PSUM") as gps:
                    # ---- B2: gi bias tiles: biasg[:, g] = shift @ w_ih_x.T |g + bgx[:, g]
                    biasg = consts.tile([128, 6], mybir.dt.float32)
                    pc = gps.tile([128, 6], mybir.dt.float32, tag="pg")
                    for g in range(6):
                        for k in range(8):
                            nc.tensor.matmul(pc[:, g:g + 1],
                                             wix[:, k * 768 + GOFF[g]: k * 768 + GOFF[g] + 128],
                                             shft[:, k:k + 1],
                                             start=(k == 0), stop=(k == 7),
                                             skip_group_check=True)
                    for g in range(6):
                        nc.vector.tensor_add(biasg[:, g:g + 1], pc[:, g:g + 1], bgxt[:, g:g + 1])

                    # ---- B4: gi_m into column 32 of each gate block (needed from t=0) ----
                    for g in range(6):
                        pg = gps.tile([128, T], mybir.dt.float32, tag="pg")
                        for k in range(8):
                            nc.tensor.matmul(pg[:], wim[:, k * 768 + GOFF[g]: k * 768 + GOFF[g] + 128],
                                             memt[:, k * T:(k + 1) * T],
                                             start=(k == 0), stop=(k == 7))
                        src = pg[:].rearrange("p (t b) -> p t b", b=1)
                        dst = gi3[:, :, COFF[g] + BS:COFF[g] + BS + 1]
                        nc.scalar.activation(dst, src, AF.Identity,
                                             bias=bgmt[:, g:g + 1], scale=1.0)

                    # ---- B3 + scan interleaved: chunk ch feeds steps 16ch..16ch+16 ----
                    NCH = 8
                    CH = BS * T // NCH                      # 512
                    TCH = CH // BS                          # 16 timesteps per chunk
                    for ch in range(NCH):
                        e16 = work.tile([128, 8 * CH], f16, tag="e16")
                        for m in range(8):
                            pe = pps.tile([128, CH], mybir.dt.float32)
                            nc.tensor.matmul(pe[:], wemb[:, m * 128:(m + 1) * 128],
                                             xt[:, ch * CH:(ch + 1) * CH], start=True, stop=True)
                            if zero_bemb:
                                rl = work.tile([128, CH], mybir.dt.float32, tag="rl")
                                nc.scalar.activation(rl[:], pe[:], AF.Relu, bias=0.0, scale=0.8)
                                nc.vector.scalar_tensor_tensor(e16[:, m * CH:(m + 1) * CH],
                                                               pe[:], 0.2, rl[:],
                                                               op0=OP.mult, op1=OP.add)
                            else:
                                lin = work.tile([128, CH], mybir.dt.float32, tag="rl")
                                nc.scalar.activation(lin[:], pe[:], AF.Identity,
                                                     bias=bembt[:, m:m + 1], scale=1.0)
                                nc.vector.scalar_tensor_tensor(e16[:, m * CH:(m + 1) * CH],
                                                               lin[:], 0.2, lin[:],
                                                               op0=OP.mult, op1=OP.max)
                        for g in range(6):
                            pg = gps.tile([128, CH], mybir.dt.float32, tag="pg")
                            for k in range(8):
                                nc.tensor.matmul(pg[:], W2[:, k * 768 + GOFF[g]: k * 768 + GOFF[g] + 128],
                                                 e16[:, k * CH:(k + 1) * CH],
                                                 start=(k == 0), stop=(k == 7))
                            src = pg[:].rearrange("p (t b) -> p t b", b=BS)
                            dst = gi3[:, ch * TCH:(ch + 1) * TCH, COFF[g]:COFF[g] + BS]
                            nc.scalar.activation(dst, src, AF.Identity,
                                                 bias=biasg[:, g:g + 1], scale=1.0)
                        for t in range(ch * TCH, (ch + 1) * TCH):
                            scan_step(t)

            # ---- final gate ----
            with tc.tile_pool(name="fin", bufs=1, space="PSUM") as finp:
                pq = finp.tile([SIM, 34], mybir.dt.float32, tag="pq")
                for k in range(2):
                    nc.tensor.matmul(pq[:, 0:33], wsx[:, k * SIM:(k + 1) * SIM],
                                     hxm[:, k * 33:(k + 1) * 33],
                                     start=(k == 0), stop=False, skip_group_check=True)
                for k in range(2):
                    nc.tensor.matmul(pq[:, 33:34], wsm[:, k * SIM:(k + 1) * SIM],
                                     hxm[:, k * 33 + 32:k * 33 + 33],
                                     start=False, stop=(k == 1), skip_group_check=True)
                q_sb = work.tile([SIM, 34], mybir.dt.float32, tag="qsb")
                nc.scalar.activation(q_sb[:, 0:33], pq[:, 0:33], AF.Identity,
                                     bias=bst[:, 0:1], scale=1.0)
                nc.scalar.activation(q_sb[:, 33:34], pq[:, 33:34], AF.Identity,
                                     bias=bst[:, 1:2], scale=1.0)
                p_sb = work.tile([SIM, 67], mybir.dt.float32, tag="psb")
                nc.vector.tensor_scalar_mul(p_sb[:, 0:33], q_sb[:, 0:33], q_sb[:, 33:34])
                nc.vector.tensor_mul(p_sb[:, 33:67], q_sb[:], q_sb[:])
                cs = finp.tile([1, 67], mybir.dt.float32, tag="cs")
                nc.tensor.matmul(cs[:], o4[:], p_sb[:], start=True, stop=True,
                                 skip_group_check=True)
                s_sb = work.tile([1, 34], mybir.dt.float32, tag="ssb")
                nc.scalar.activation(s_sb[:], cs[0:1, 33:67], AF.Sqrt)
                nc.vector.tensor_scalar_max(s_sb[:], s_sb[:], EPS_COS)
                den = work.tile([1, 33], mybir.dt.float32, tag="den")
                nc.vector.tensor_scalar_mul(den[:], s_sb[:, 0:33], s_sb[:, 33:34])
                nc.vector.reciprocal(den[:], den[:])
                rat = work.tile([1, 33], mybir.dt.float32, tag="rat")
                nc.vector.tensor_mul(rat[:], cs[0:1, 0:33], den[:])
                g_sb = work.tile([1, 33], mybir.dt.float32, tag="gsb")
                nc.scalar.activation(g_sb[:], rat[:], AF.Sigmoid)
                gbc = finp.tile([128, BS], mybir.dt.float32, tag="gbc")
                nc.tensor.matmul(gbc[:], o128[:], g_sb[0:1, 0:BS], start=True, stop=True,
                                 skip_group_check=True)
                hm32 = work.tile([128, 2], mybir.dt.float32, tag="hm32")
                hmv = hxm[:].rearrange("p (k c) -> p k c", c=33)[:, :, 32:33]
                nc.scalar.activation(hm32[:].rearrange("p (k c) -> p k c", c=1), hmv,
                                     AF.Identity, bias=0.0, scale=1.0)
                out_sb = work.tile([128, 2, BS], mybir.dt.float32, tag="outsb")
                for k in range(2):
                    ddk = work.tile([128, BS], mybir.dt.float32, tag="ddk")
                    nc.vector.tensor_scalar_sub(ddk[:], hxm[:, k * 33:k * 33 + 32],
                                                hm32[:, k:k + 1])
                    ppk = work.tile([128, BS], mybir.dt.float32, tag="ppk")
                    nc.vector.tensor_mul(ppk[:], gbc[:], ddk[:])
                    nc.vector.tensor_scalar_add(out_sb[:, k, :], ppk[:], hm32[:, k:k + 1])
                nc.sync.dma_start(out=outT[:], in_=out_sb[:])
    nc.finalize()
    return nc


def _prep_inputs(inputs):
    x = _f32(inputs["x"])
    W_emb = _f32(inputs["W_emb"]); b_emb = _f32(inputs["b_emb"])
    gamma = _f32(inputs["gamma"]); beta = _f32(inputs["beta"])
    mem = _f32(inputs["memory"])[0]
    w_ih_x = _f32(inputs["w_ih_x"]); w_hh_x = _f32(inputs["w_hh_x"])
    b_ih_x = _f32(inputs["b_ih_x"]); b_hh_x = _f32(inputs["b_hh_x"])
    w_ih_m = _f32(inputs["w_ih_m"]); w_hh_m = _f32(inputs["w_hh_m"])
    b_ih_m = _f32(inputs["b_ih_m"]); b_hh_m = _f32(inputs["b_hh_m"])
    W_sx = _f32(inputs["W_sx"]); b_sx = _f32(inputs["b_sx"])
    W_sm = _f32(inputs["W_sm"]); b_sm = _f32(inputs["b_sm"])

    zero_bemb = not np.any(b_emb)
    has_bhn = bool(np.any(b_hh_x[512:]) or np.any(b_hh_m[512:]))

    per_core = []
    WembT = _f16(W_emb.T)
    bemb_t = _f32(b_emb.reshape(8, 128).T)
    for c in range(NCORES):
        xc = x[c * BS:(c + 1) * BS]                       # (32,T,IN)
        xTc = _f16(xc.transpose(2, 1, 0).reshape(IN, T * BS))
        per_core.append({"xT": xTc, "WembT": WembT, "bemb": bemb_t})

    shared = dict(
        WembT=WembT, bemb=bemb_t,
        wihxT=_f16(_ktile(w_ih_x.T, 8, 768)),
        wihmT=_f16(_ktile(w_ih_m.T, 8, 768)),
        memT=_f16(_ktile(mem.T, 8, T)),
        whhxT=_f16(_ktile(w_hh_x.T, 2, 768)),
        whhmT=_f16(_ktile(w_hh_m.T, 2, 768)),
        ident=_f16(np.eye(128)),
        WsxT=_f16(_ktile(W_sx.T, 2, SIM)),
        WsmT=_f16(_ktile(W_sm.T, 2, SIM)),
        bs4=_f32(np.stack([b_sx, b_sm], axis=1)),
        ones4=_f32(np.ones((SIM, 1))),
        ones128=_f32(np.ones((1, 128))),
    )
    bgx = np.empty((128, 6), np.float32)
    bgm = np.empty((128, 6), np.float32)
    for g in range(6):
        sl = slice(GOFF[g], GOFF[g] + 128)
        if g < 4:
            bgx[:, g] = b_ih_x[sl] + b_hh_x[sl]
            bgm[:, g] = b_ih_m[sl] + b_hh_m[sl]
        else:
            bgx[:, g] = b_ih_x[sl]
            bgm[:, g] = b_ih_m[sl]
    shared["bgx"] = _f32(bgx); shared["bgm"] = _f32(bgm)
    if has_bhn:
        shared["bhn"] = _f32(np.concatenate([b_hh_x[512:768], b_hh_m[512:768]]).reshape(1, -1))
        shared["onesb"] = _f32(np.ones((1, BS)))
    meta = dict(zero_bemb=zero_bemb, has_bhn=has_bhn, gamma=gamma, beta=beta)
    return per_core, shared, meta


def _combine_stats(bn_results, gamma, beta):
    # bn_results: list of (128,8,2) per core; channel h = m*128+p
    means = np.stack([r.reshape(128, 8, 2)[:, :, 0].T.reshape(H) for r in bn_results])
    vars_ = np.stack([r.reshape(128, 8, 2)[:, :, 1].T.reshape(H) for r in bn_results])
    mean = means.mean(0)
    var = (vars_ + means ** 2).mean(0) - mean ** 2
    scale = gamma / np.sqrt(var + EPS_BN)
    shift = beta - mean * scale
    return scale, shift


def get_programs(zero_bemb, has_bhn):
    key = ("progs", zero_bemb, has_bhn)
    if key not in _cache:
        _cache[key] = (_build_phase_a(zero_bemb), _build_phase_b(zero_bemb, has_bhn))
    return _cache[key]


def kernel(**inputs) -> np.ndarray:
    from concourse.bass_utils import run_bass_kernel_spmd

    per_core, shared, meta = _prep_inputs(inputs)
    nc_a, nc_b = get_programs(meta["zero_bemb"], meta["has_bhn"])
    core_ids = list(range(NCORES))

    in_a = [{"xT": pc["xT"], "WembT": pc["WembT"], "bemb": pc["bemb"]}
            for pc in per_core]
    res_a = run_bass_kernel_spmd(nc_a, in_a, core_ids=core_ids).results
    scale, shift = _combine_stats([r["bnout"] for r in res_a],
                                  meta["gamma"], meta["beta"])

    scl_t = _f32(scale.reshape(8, 128).T)
    shf_t = _f16(shift.reshape(8, 128).T)
    in_b = []
    for c in range(NCORES):
        m = {"xT": per_core[c]["xT"], "scl": scl_t, "shf16": shf_t}
        m.update(shared)
        in_b.append(m)
    res_b = run_bass_kernel_spmd(nc_b, in_b, core_ids=core_ids).results

    out = np.empty((B, OUT), np.float32)
    for c in range(NCORES):
        o = res_b[c]["outT"].reshape(128, 2, BS)
        out[c * BS:(c + 1) * BS] = o.transpose(2, 1, 0).reshape(BS, OUT)
    return out


# revision 11
# speedup vs baseline: 3521.9708x; 3463.8187x over previous
"""Trainium2 Bass kernel for nn_MemoryDiscriminator.

Strategy (data-parallel over batch, 8 cores, 32 rows each):
  - Only outs[-1] of the reference scan is used -> no per-step outputs needed.
  - The memory-GRU state hm is batch-independent -> carried as a 33rd batch
    column through the same PSUM tiles / elementwise ops as hx.
  - gi_x = BN(e) @ w_ih_x.T is precomputed for all T as one big matmul with
    the BN affine folded into the weights (scale per contraction-dim
    partition, shift folded into the bias via a small matvec).
  - BatchNorm statistics need a global (B,T) reduction: phase A computes
    per-core per-channel mean/var on device (bn_stats/bn_aggr); the host
    combines 8x(1024,2) scalars and feeds scale/shift to phase B.
  - Transposed layout throughout: features on partitions, batch on free dim.
"""

import numpy as np

B, T, IN, H, OUT, SIM = 256, 128, 128, 1024, 256, 4
NCORES, BS = 8, 32
GOFF = [0, 128, 256, 384, 512, 640]          # gate-dim offset per tile: r0 r1 z0 z1 n0 n1
COFF = [0, 33, 66, 99, 132, 165]             # column offset of each gate block in gi row
STEP = 6 * 33                                # 198 columns per timestep in gi
EPS_BN, EPS_COS = 1e-5, 1e-8

_cache = {}


def _f16(a):
    return np.ascontiguousarray(a, dtype=np.float16)


def _f32(a):
    return np.ascontiguousarray(a, dtype=np.float32)


def _ktile(a, k, n):
    # (k*128, n) -> (128, k*n) with column-block j = rows j*128..j*128+128
    return np.ascontiguousarray(a.reshape(k, 128, n).transpose(1, 0, 2).reshape(128, k * n))


def _build_phase_a(zero_bemb):
    from concourse import bacc
    import concourse.mybir as mybir
    import concourse.tile as tile

    f32, f16 = mybir.dt.float32, mybir.dt.float16
    AF = mybir.ActivationFunctionType
    OP = mybir.AluOpType

    nc = bacc.Bacc()
    xT = nc.declare_dram_parameter("xT", [128, BS * T], f16, isOutput=False)
    WembT = nc.declare_dram_parameter("WembT", [128, H], f16, isOutput=False)
    bemb = nc.declare_dram_parameter("bemb", [128, 8], f32, isOutput=False)
    bnout = nc.declare_dram_parameter("bnout", [128, 8, 2], f32, isOutput=True)

    with tile.TileContext(nc) as tc:
        with tc.tile_pool(name="consts", bufs=1) as consts, \
             tc.tile_pool(name="work", bufs=3) as work, \
             tc.tile_pool(name="stat", bufs=1) as statp, \
             tc.tile_pool(name="ps", bufs=3, space="PSUM") as psum:
            xt = consts.tile([128, BS * T], f16)
            wt = consts.tile([128, H], f16)
            nc.sync.dma_start(out=xt[:], in_=xT[:])
            nc.sync.dma_start(out=wt[:], in_=WembT[:])
            if not zero_bemb:
                bt = consts.tile([128, 8], f32)
                nc.sync.dma_start(out=bt[:], in_=bemb[:])
            out_sb = statp.tile([128, 8, 2], mybir.dt.float32)
            NCH = 8
            CH = BS * T // NCH                     # 512
            for m in range(8):
                stats = statp.tile([128, NCH, 6], mybir.dt.float32)
                for ch in range(NCH):
                    pe = psum.tile([128, CH], mybir.dt.float32)
                    nc.tensor.matmul(pe[:], wt[:, m * 128:(m + 1) * 128],
                                     xt[:, ch * CH:(ch + 1) * CH], start=True, stop=True)
                    e_sb = work.tile([128, CH], mybir.dt.float32)
                    if zero_bemb:
                        rl = work.tile([128, CH], mybir.dt.float32)
                        nc.scalar.activation(rl[:], pe[:], AF.Relu, bias=0.0, scale=0.8)
                        nc.vector.scalar_tensor_tensor(e_sb[:], pe[:], 0.2, rl[:],
                                                       op0=OP.mult, op1=OP.add)
                    else:
                        lin = work.tile([128, CH], mybir.dt.float32)
                        nc.scalar.activation(lin[:], pe[:], AF.Identity,
                                             bias=bt[:, m:m + 1], scale=1.0)
                        nc.vector.scalar_tensor_tensor(e_sb[:], lin[:], 0.2, lin[:],
                                                       op0=OP.mult, op1=OP.max)
                    nc.vector.bn_stats(stats[:, ch, :], e_sb[:])
                nc.vector.bn_aggr(out_sb[:, m, :], stats[:])
            nc.sync.dma_start(out=bnout[:], in_=out_sb[:])
    nc.finalize()
    return nc


def _build_phase_b(zero_bemb, has_bhn):
    from concourse import bacc
    import concourse.mybir as mybir
    import concourse.tile as tile

    f32, f16 = mybir.dt.float32, mybir.dt.float16
    AF = mybir.ActivationFunctionType
    OP = mybir.AluOpType

    nc = bacc.Bacc()
    dp = nc.declare_dram_parameter
    xT = dp("xT", [128, BS * T], f16, isOutput=False)
    WembT = dp("WembT", [128, H], f16, isOutput=False)
    bemb = dp("bemb", [128, 8], f32, isOutput=False)
    scl = dp("scl", [128, 8], f32, isOutput=False)       # BN scale per channel
    shf16 = dp("shf16", [128, 8], f16, isOutput=False)   # BN shift (fp16)
    wihxT = dp("wihxT", [128, 8 * 768], f16, isOutput=False)
    bgx = dp("bgx", [128, 6], f32, isOutput=False)
    wihmT = dp("wihmT", [128, 8 * 768], f16, isOutput=False)
    bgm = dp("bgm", [128, 6], f32, isOutput=False)
    memT = dp("memT", [128, 8 * T], f16, isOutput=False)
    whhxT = dp("whhxT", [128, 2 * 768], f16, isOutput=False)
    whhmT = dp("whhmT", [128, 2 * 768], f16, isOutput=False)
    ident = dp("ident", [128, 128], f16, isOutput=False)
    WsxT = dp("WsxT", [128, 2 * SIM], f16, isOutput=False)
    WsmT = dp("WsmT", [128, 2 * SIM], f16, isOutput=False)
    bs4 = dp("bs4", [SIM, 2], f32, isOutput=False)       # col0 = b_sx, col1 = b_sm
    ones4 = dp("ones4", [SIM, 1], f32, isOutput=False)
    ones128 = dp("ones128", [1, 128], f32, isOutput=False)
    if has_bhn:
        bhn = dp("bhn", [1, 512], f32, isOutput=False)  # [bhh_x_n (256) | bhh_m_n (256)]
        onesb = dp("onesb", [1, BS], f32, isOutput=False)
    outT = dp("outT", [128, 2, BS], f32, isOutput=True)

    with tile.TileContext(nc) as tc:
        with tc.tile_pool(name="consts", bufs=1) as consts, \
             tc.tile_pool(name="gi", bufs=1) as gip, \
             tc.tile_pool(name="state", bufs=3) as statep, \
             tc.tile_pool(name="work", bufs=3) as work:
            # ---- load inputs ----
            def cl(t_, src):
                tt = consts.tile(list(src.shape), src.dtype, tag=t_)
                nc.sync.dma_start(out=tt[:], in_=src[:])
                return tt

            xt = cl("xt", xT); wemb = cl("wemb", WembT)
            sclt = cl("sclt", scl); shft = cl("shft", shf16)
            wix = cl("wix", wihxT); bgxt = cl("bgxt", bgx)
            wim = cl("wim", wihmT); bgmt = cl("bgmt", bgm)
            memt = cl("memt", memT)
            whx = cl("whx", whhxT); whm = cl("whm", whhmT)
            idt = cl("idt", ident)
            wsx = cl("wsx", WsxT); wsm = cl("wsm", WsmT)
            bst = cl("bst", bs4); o4 = cl("o4", ones4); o128 = cl("o128", ones128)
            if not zero_bemb:
                bembt = cl("bembt", bemb)
            if has_bhn:
                bhnt = cl("bhnt", bhn); onbt = cl("onbt", onesb)

            gi_sb = gip.tile([128, T * STEP], f16)      # (128, 25344)
            gi3 = gi_sb[:].rearrange("p (t c) -> p t c", c=STEP)

            # ---- B1: fold W2 = wihxT * scale (per-partition) ----
            W2 = consts.tile([128, 8 * 768], f16)
            for k in range(8):
                nc.vector.tensor_scalar_mul(W2[:, k * 768:(k + 1) * 768],
                                            wix[:, k * 768:(k + 1) * 768],
                                            sclt[:, k:k + 1])

            with tc.tile_pool(name="rzp", bufs=2, space="PSUM") as rzp, \
                 tc.tile_pool(name="npp", bufs=2, space="PSUM") as npp:
                hxm = statep.tile([128, 66], f16, tag="hxm")
                nc.gpsimd.memset(hxm[:], 0.0)

                def scan_step(t):
                    nonlocal hxm
                    prz = rzp.tile([128, 132], mybir.dt.float32)
                    pn = npp.tile([128, 66], mybir.dt.float32)
                    # identity first (no data dependence on state): start=True clears bank
                    nc.tensor.matmul(prz[:], idt[:], gi3[:, t, 0:132],
                                     start=True, stop=False, skip_group_check=True)
                    # k-major: all k=0 matmuls first (they only need hxm cols 0:33)
                    first_n = True
                    for k in range(2):
                        for g in range(4):
                            nc.tensor.matmul(prz[:, g * 33:g * 33 + 32],
                                             whx[:, k * 768 + GOFF[g]: k * 768 + GOFF[g] + 128],
                                             hxm[:, k * 33:k * 33 + 32],
                                             start=False, stop=False, skip_group_check=True)
                            nc.tensor.matmul(prz[:, g * 33 + 32:g * 33 + 33],
                                             whm[:, k * 768 + GOFF[g]: k * 768 + GOFF[g] + 128],
                                             hxm[:, k * 33 + 32:k * 33 + 33],
                                             start=False, stop=(g == 3 and k == 1),
                                             skip_group_check=True)
                        for g in range(4, 6):
                            gg = g - 4
                            nc.tensor.matmul(pn[:, gg * 33:gg * 33 + 32],
                                             whx[:, k * 768 + GOFF[g]: k * 768 + GOFF[g] + 128],
                                             hxm[:, k * 33:k * 33 + 32],
                                             start=first_n, stop=False, skip_group_check=True)
                            first_n = False
                            nc.tensor.matmul(pn[:, gg * 33 + 32:gg * 33 + 33],
                                             whm[:, k * 768 + GOFF[g]: k * 768 + GOFF[g] + 128],
                                             hxm[:, k * 33 + 32:k * 33 + 33],
                                             start=False, stop=(g == 5 and k == 1),
                                             skip_group_check=True)
                    if has_bhn:
                        for kk in range(2):
                            nc.tensor.matmul(pn[:, kk * 33:kk * 33 + 32],
                                             bhnt[0:1, kk * 128:(kk + 1) * 128],
                                             onbt[0:1, :], start=False, stop=False,
                                             skip_group_check=True)
                            nc.tensor.matmul(pn[:, kk * 33 + 32:kk * 33 + 33],
                                             bhnt[0:1, 256 + kk * 128:256 + (kk + 1) * 128],
                                             onbt[0:1, 0:1], start=False, stop=False,
                                             skip_group_check=True)
                    # critical chain: sig_r -> t1 -> t2 -> tanh -> nz -> hxm'
                    rs = work.tile([128, 66], mybir.dt.float32, tag="rs")
                    nc.scalar.activation(rs[:], prz[:, 0:66], AF.Sigmoid)
                    zc = work.tile([128, 66], mybir.dt.float32, tag="zc")
                    nc.scalar.activation(zc[:], prz[:, 66:132], AF.Sigmoid, scale=-1.0)
                    t1 = work.tile([128, 66], mybir.dt.float32, tag="t1")
                    nc.vector.tensor_mul(t1[:], rs[:], pn[:])
                    t2 = work.tile([128, 66], mybir.dt.float32, tag="t2")
                    nc.vector.tensor_add(t2[:], t1[:], gi3[:, t, 132:198])
                    # off-chain: u = hxm - zc*hxm  (runs during tanh)
                    zh = work.tile([128, 66], mybir.dt.float32, tag="zh")
                    nc.vector.tensor_mul(zh[:], zc[:], hxm[:])
                    u = work.tile([128, 66], mybir.dt.float32, tag="u")
                    nc.vector.tensor_sub(u[:], hxm[:], zh[:])
                    n_sb = work.tile([128, 66], mybir.dt.float32, tag="nsb")
                    nc.scalar.activation(n_sb[:], t2[:], AF.Tanh)
                    nz = work.tile([128, 66], mybir.dt.float32, tag="nz")
                    nc.vector.tensor_mul(nz[:], zc[:], n_sb[:])
                    hxm = statep.tile([128, 66], f16, tag="hxm")
                    nc.vector.tensor_add(hxm[:], u[:], nz[:])

                with tc.tile_pool(name="pps", bufs=2, space="PSUM") as pps, \
                     tc.tile_pool(name="gps", bufs=2, space="PSUM") as gps:
                    # ---- B2: gi bias tiles: biasg[:, g] = shift @ w_ih_x.T |g + bgx[:, g]
                    biasg = consts.tile([128, 6], mybir.dt.float32)
                    pc = gps.tile([128, 6], mybir.dt.float32, tag="pg")
                    for g in range(6):
                        for k in range(8):
                            nc.tensor.matmul(pc[:, g:g + 1],
                                             wix[:, k * 768 + GOFF[g]: k * 768 + GOFF[g] + 128],
                                             shft[:, k:k + 1],
                                             start=(k == 0), stop=(k == 7),
                                             skip_group_check=True)
                    for g in range(6):
                        nc.vector.tensor_add(biasg[:, g:g + 1], pc[:, g:g + 1], bgxt[:, g:g + 1])

                    # ---- B4: gi_m into column 32 of each gate block (needed from t=0) ----
                    for g in range(6):
                        pg = gps.tile([128, T], mybir.dt.float32, tag="pg")
                        for k in range(8):
                            nc.tensor.matmul(pg[:], wim[:, k * 768 + GOFF[g]: k * 768 + GOFF[g] + 128],
                                             memt[:, k * T:(k + 1) * T],
                                             start=(k == 0), stop=(k == 7))
                        src = pg[:].rearrange("p (t b) -> p t b", b=1)
                        dst = gi3[:, :, COFF[g] + BS:COFF[g] + BS + 1]
                        nc.scalar.activation(dst, src, AF.Identity,
                                             bias=bgmt[:, g:g + 1], scale=1.0)

                    # ---- B3 + scan interleaved: chunk ch feeds steps 16ch..16ch+16 ----
                    NCH = 8
                    CH = BS * T // NCH                      # 512
                    TCH = CH // BS                          # 16 timesteps per chunk
                    for ch in range(NCH):
                        e16 = work.tile([128, 8 * CH], f16, tag="e16")
                        for m in range(8):
                            pe = pps.tile([128, CH], mybir.dt.float32)
                            nc.tensor.matmul(pe[:], wemb[:, m * 128:(m + 1) * 128],
                                             xt[:, ch * CH:(ch + 1) * CH], start=True, stop=True)
                            if zero_bemb:
                                rl = work.tile([128, CH], mybir.dt.float32, tag="rl")
                                nc.scalar.activation(rl[:], pe[:], AF.Relu, bias=0.0, scale=0.8)
                                nc.vector.scalar_tensor_tensor(e16[:, m * CH:(m + 1) * CH],
                                                               pe[:], 0.2, rl[:],
                                                               op0=OP.mult, op1=OP.add)
                            else:
                                lin = work.tile([128, CH], mybir.dt.float32, tag="rl")
                                nc.scalar.activation(lin[:], pe[:], AF.Identity,
                                                     bias=bembt[:, m:m + 1], scale=1.0)
                                nc.vector.scalar_tensor_tensor(e16[:, m * CH:(m + 1) * CH],
                                                               lin[:], 0.2, lin[:],
                                                               op0=OP.mult, op1=OP.max)
                        for g in range(6):
                            pg = gps.tile([128, CH], mybir.dt.float32, tag="pg")
                            for k in range(8):
                                nc.tensor.matmul(pg[:], W2[:, k * 768 + GOFF[g]: k * 768 + GOFF[g] + 128],
                                                 e16[:, k * CH:(k + 1) * CH],
                                                 start=(k == 0), stop=(k == 7))
                            src = pg[:].rearrange("p (t b) -> p t b", b=BS)
                            dst = gi3[:, ch * TCH:(ch + 1) * TCH, COFF[g]:COFF[g] + BS]
                            nc.scalar.activation(dst, src, AF.Identity,
                                                 bias=biasg[:, g:g + 1], scale=1.0)
                        for t in range(ch * TCH, (ch + 1) * TCH):
                            scan_step(t)

            # ---- final gate ----
            with tc.tile_pool(name="fin", bufs=1, space="PSUM") as finp:
                pq = finp.tile([SIM, 34], mybir.dt.float32, tag="pq")
                for k in range(2):
                    nc.tensor.matmul(pq[:, 0:33], wsx[:, k * SIM:(k + 1) * SIM],
                                     hxm[:, k * 33:(k + 1) * 33],
                                     start=(k == 0), stop=False, skip_group_check=True)
                for k in range(2):
                    nc.tensor.matmul(pq[:, 33:34], wsm[:, k * SIM:(k + 1) * SIM],
                                     hxm[:, k * 33 + 32:k * 33 + 33],
                                     start=False, stop=(k == 1), skip_group_check=True)
                q_sb = work.tile([SIM, 34], mybir.dt.float32, tag="qsb")
                nc.scalar.activation(q_sb[:, 0:33], pq[:, 0:33], AF.Identity,
                                     bias=bst[:, 0:1], scale=1.0)
                nc.scalar.activation(q_sb[:, 33:34], pq[:, 33:34], AF.Identity,
                                     bias=bst[:, 1:2], scale=1.0)
                p_sb = work.tile([SIM, 67], mybir.dt.float32, tag="psb")
                nc.vector.tensor_scalar_mul(p_sb[:, 0:33], q_sb[:, 0:33], q_sb[:, 33:34])
                nc.vector.tensor_mul(p_sb[:, 33:67], q_sb[:], q_sb[:])
                cs = finp.tile([1, 67], mybir.dt.float32, tag="cs")
                nc.tensor.matmul(cs[:], o4[:], p_sb[:], start=True, stop=True,
                                 skip_group_check=True)
                s_sb = work.tile([1, 34], mybir.dt.float32, tag="ssb")
                nc.scalar.activation(s_sb[:], cs[0:1, 33:67], AF.Sqrt)
                nc.vector.tensor_scalar_max(s_sb[:], s_sb[:], EPS_COS)
                den = work.tile([1, 33], mybir.dt.float32, tag="den")
                nc.vector.tensor_scalar_mul(den[:], s_sb[:, 0:33], s_sb[:, 33:34])
                nc.vector.reciprocal(den[:], den[:])
                rat = work.tile([1, 33], mybir.dt.float32, tag="rat")
                nc.vector.tensor_mul(rat[:], cs[0:1, 0:33], den[:])
                g_sb = work.tile([1, 33], mybir.dt.float32, tag="gsb")
                nc.scalar.activation(g_sb[:], rat[:], AF.Sigmoid)
                gbc = finp.tile([128, BS], mybir.dt.float32, tag="gbc")
                nc.tensor.matmul(gbc[:], o128[:], g_sb[0:1, 0:BS], start=True, stop=True,
                                 skip_group_check=True)
                hm32 = work.tile([128, 2], mybir.dt.float32, tag="hm32")
                hmv = hxm[:].rearrange("p (k c) -> p k c", c=33)[:, :, 32:33]
                nc.scalar.activation(hm32[:].rearrange("p (k c) -> p k c", c=1), hmv,
                                     AF.Identity, bias=0.0, scale=1.0)
                out_sb = work.tile([128, 2, BS], mybir.dt.float32, tag="outsb")
                for k in range(2):
                    ddk = work.tile([128, BS], mybir.dt.float32, tag="ddk")
                    nc.vector.tensor_scalar_sub(ddk[:], hxm[:, k * 33:k * 33 + 32],
                                                hm32[:, k:k + 1])
                    ppk = work.tile([128, BS], mybir.dt.float32, tag="ppk")
                    nc.vector.tensor_mul(ppk[:], gbc[:], ddk[:])
                    nc.vector.tensor_scalar_add(out_sb[:, k, :], ppk[:], hm32[:, k:k + 1])
                nc.sync.dma_start(out=outT[:], in_=out_sb[:])
    nc.finalize()
    return nc


def _prep_inputs(inputs):
    x = _f32(inputs["x"])
    W_emb = _f32(inputs["W_emb"]); b_emb = _f32(inputs["b_emb"])
    gamma = _f32(inputs["gamma"]); beta = _f32(inputs["beta"])
    mem = _f32(inputs["memory"])[0]
    w_ih_x = _f32(inputs["w_ih_x"]); w_hh_x = _f32(inputs["w_hh_x"])
    b_ih_x = _f32(inputs["b_ih_x"]); b_hh_x = _f32(inputs["b_hh_x"])
    w_ih_m = _f32(inputs["w_ih_m"]); w_hh_m = _f32(inputs["w_hh_m"])
    b_ih_m = _f32(inputs["b_ih_m"]); b_hh_m = _f32(inputs["b_hh_m"])
    W_sx = _f32(inputs["W_sx"]); b_sx = _f32(inputs["b_sx"])
    W_sm = _f32(inputs["W_sm"]); b_sm = _f32(inputs["b_sm"])

    zero_bemb = not np.any(b_emb)
    has_bhn = bool(np.any(b_hh_x[512:]) or np.any(b_hh_m[512:]))

    per_core = []
    WembT = _f16(W_emb.T)
    bemb_t = _f32(b_emb.reshape(8, 128).T)
    for c in range(NCORES):
        xc = x[c * BS:(c + 1) * BS]                       # (32,T,IN)
        xTc = _f16(xc.transpose(2, 1, 0).reshape(IN, T * BS))
        per_core.append({"xT": xTc, "WembT": WembT, "bemb": bemb_t})

    shared = dict(
        WembT=WembT, bemb=bemb_t,
        wihxT=_f16(_ktile(w_ih_x.T, 8, 768)),
        wihmT=_f16(_ktile(w_ih_m.T, 8, 768)),
        memT=_f16(_ktile(mem.T, 8, T)),
        whhxT=_f16(_ktile(w_hh_x.T, 2, 768)),
        whhmT=_f16(_ktile(w_hh_m.T, 2, 768)),
        ident=_f16(np.eye(128)),
        WsxT=_f16(_ktile(W_sx.T, 2, SIM)),
        WsmT=_f16(_ktile(W_sm.T, 2, SIM)),
        bs4=_f32(np.stack([b_sx, b_sm], axis=1)),
        ones4=_f32(np.ones((SIM, 1))),
        ones128=_f32(np.ones((1, 128))),
    )
    bgx = np.empty((128, 6), np.float32)
    bgm = np.empty((128, 6), np.float32)
    for g in range(6):
        sl = slice(GOFF[g], GOFF[g] + 128)
        if g < 4:
            bgx[:, g] = b_ih_x[sl] + b_hh_x[sl]
            bgm[:, g] = b_ih_m[sl] + b_hh_m[sl]
        else:
            bgx[:, g] = b_ih_x[sl]
            bgm[:, g] = b_ih_m[sl]
    shared["bgx"] = _f32(bgx); shared["bgm"] = _f32(bgm)
    if has_bhn:
        shared["bhn"] = _f32(np.concatenate([b_hh_x[512:768], b_hh_m[512:768]]).reshape(1, -1))
        shared["onesb"] = _f32(np.ones((1, BS)))
    meta = dict(zero_bemb=zero_bemb, has_bhn=has_bhn, gamma=gamma, beta=beta)
    return per_core, shared, meta


def _combine_stats(bn_results, gamma, beta):
    # bn_results: list of (128,8,2) per core; channel h = m*128+p
    means = np.stack([r.reshape(128, 8, 2)[:, :, 0].T.reshape(H) for r in bn_results])
    vars_ = np.stack([r.reshape(128, 8, 2)[:, :, 1].T.reshape(H) for r in bn_results])
    mean = means.mean(0)
    var = (vars_ + means ** 2).mean(0) - mean ** 2
    scale = gamma / np.sqrt(var + EPS_BN)
    shift = beta - mean * scale
    return scale, shift


def get_programs(zero_bemb, has_bhn):
    key = ("progs", zero_bemb, has_bhn)
    if key not in _cache:
        _cache[key] = (_build_phase_a(zero_bemb), _build_phase_b(zero_bemb, has_bhn))
    return _cache[key]


def kernel(**inputs) -> np.ndarray:
    from concourse.bass_utils import run_bass_kernel_spmd

    per_core, shared, meta = _prep_inputs(inputs)
    nc_a, nc_b = get_programs(meta["zero_bemb"], meta["has_bhn"])
    core_ids = list(range(NCORES))

    in_a = [{"xT": pc["xT"], "WembT": pc["WembT"], "bemb": pc["bemb"]}
            for pc in per_core]
    res_a = run_bass_kernel_spmd(nc_a, in_a, core_ids=core_ids).results
    scale, shift = _combine_stats([r["bnout"] for r in res_a],
                                  meta["gamma"], meta["beta"])

    scl_t = _f32(scale.reshape(8, 128).T)
    shf_t = _f16(shift.reshape(8, 128).T)
    in_b = []
    for c in range(NCORES):
        m = {"xT": per_core[c]["xT"], "scl": scl_t, "shf16": shf_t}
        m.update(shared)
        in_b.append(m)
    res_b = run_bass_kernel_spmd(nc_b, in_b, core_ids=core_ids).results

    out = np.empty((B, OUT), np.float32)
    for c in range(NCORES):
        o = res_b[c]["outT"].reshape(128, 2, BS)
        out[c * BS:(c + 1) * BS] = o.transpose(2, 1, 0).reshape(BS, OUT)
    return out
